# revision 17
# baseline (speedup 1.0000x reference)
"""Trainium2 Bass kernel for -mean(antonymy_score > synonymy_score).

Strategy: pure data-parallel over 8 NeuronCores. Each core receives a
contiguous 1/8 slice of the batch. On the host, antonymy/synonymy are
interleaved into one flat "pairs" tensor laid out as consecutive
[128, 2, fd_j] chunk blocks so each chunk arrives in a single contiguous
DMA carrying one semaphore (the walrus codegen path here allows only one
sync wait per instruction). Chunks alternate between the two HWDGE rings
(SP + ACT) so the two streams transfer concurrently (~341 GB/s observed,
~358 GB/s per-core HBM cap). Chunk sizes taper (2MB, 1MB, 0.5MB, 0.5MB
per ring) so the DVE work left after the last chunk lands is minimal.

Each chunk is consumed by one fused DVE scalar_tensor_tensor:
mask = (ant bypass 0) is_gt syn, with accum_out producing the
per-partition free-dim sum in the same pass. The [128, n_chunks] partial
counts are DMA'd back and the host computes -total/B (exact:
integer-valued fp32 counts). S1_out is unused by the computation - it
only fixes the batch size.

Raw Bass (no TileContext) keeps the program pre/postamble to a single
all-engine barrier instead of Tile's ~10us of drains + EVSEM
butterflies. One semaphore per chunk DMA: two in-flight DMAs sharing a
sem can interleave their 16 per-SDMA-engine increments, so a cumulative
wait could fire with the earlier chunk still incomplete.
"""

from contextlib import ExitStack

import numpy as np

import concourse.bass as bass
import concourse.mybir as mybir
from concourse.bass_utils import run_bass_kernel_spmd

B = 8388608
N_CORES = 8
PER_CORE = B // N_CORES  # 1048576
P = 128
FD_TOTAL = PER_CORE // P  # 8192 per array per core

# Per-chunk (free-dim size, stream) in the order the DVE consumes them.
# Three concurrent DMA streams: the two HWDGE rings (SP + ACT) and one
# SWDGE stream (gpsimd) - the two HWDGE rings share descriptor-gen
# hardware and one of them starts ~3us late (randomly which), so SWDGE
# provides a third independent stream. Each stream tapers to a small
# final chunk so little DVE work is exposed after its stream drains.
CHUNK_PLAN = [
    (2048, "sp"),
    (2048, "act"),
    (1024, "sw"),
    (768, "sp"),
    (768, "act"),
    (640, "sw"),
    (256, "sp"),
    (256, "act"),
    (384, "sw"),
]
CHUNK_FDS = [fd for fd, _ in CHUNK_PLAN]
assert sum(CHUNK_FDS) == FD_TOTAL
N_CHUNKS = len(CHUNK_FDS)
CHUNK_OFFS = np.concatenate([[0], np.cumsum(CHUNK_FDS)]).tolist()

F32 = mybir.dt.float32

_NC = None


def build_nc():
    nc = bass.Bass()
    pairs = nc.dram_tensor("pairs", [2 * PER_CORE], F32, kind="ExternalInput")
    out = nc.dram_tensor("out", [P, N_CHUNKS], F32, kind="ExternalOutput")

    with ExitStack() as ctx:
        pair_buf = ctx.enter_context(
            nc.sbuf_tensor("pair_buf", [P, 2 * FD_TOTAL], F32)
        )
        mask_buf = ctx.enter_context(nc.sbuf_tensor("mask_buf", [P, FD_TOTAL], F32))
        partials = ctx.enter_context(nc.sbuf_tensor("partials", [P, N_CHUNKS], F32))
        chunk_sems = [
            ctx.enter_context(nc.semaphore(f"chunk{k}")) for k in range(N_CHUNKS)
        ]
        dve_sem = ctx.enter_context(nc.semaphore("dve_sem"))
        out_sem = ctx.enter_context(nc.semaphore("out_sem"))
        block = ctx.enter_context(nc.Block())

        def chunk_dma(eng, k):
            fd = CHUNK_FDS[k]
            off = CHUNK_OFFS[k]
            src = bass.AP(pairs, 2 * P * off, [[2 * fd, P], [1, 2 * fd]])
            dst = pair_buf[:, 2 * off : 2 * (off + fd)]
            eng.dma_start(dst, src).then_inc(chunk_sems[k], 16)

        @block.sync
        def _(sync: bass.BassEngine):
            for k in range(N_CHUNKS):
                if CHUNK_PLAN[k][1] == "sp":
                    chunk_dma(sync, k)
            sync.wait_ge(dve_sem, N_CHUNKS)
            sync.dma_start(out[:], partials[:]).then_inc(out_sem, 16)

        @block.scalar
        def _(scalar: bass.BassEngine):
            for k in range(N_CHUNKS):
                if CHUNK_PLAN[k][1] == "act":
                    chunk_dma(scalar, k)

        @block.gpsimd
        def _(gpsimd: bass.BassEngine):
            for k in range(N_CHUNKS):
                if CHUNK_PLAN[k][1] == "sw":
                    chunk_dma(gpsimd, k)

        @block.vector
        def _(vector: bass.BassEngine):
            for k in range(N_CHUNKS):
                fd = CHUNK_FDS[k]
                off = CHUNK_OFFS[k]
                vector.wait_ge(chunk_sems[k], 16)
                # mask = (ant bypass 0.0) is_gt syn -> 1.0/0.0
                # partials[:, k] = free-dim sum of mask (same instruction)
                vector.scalar_tensor_tensor(
                    out=mask_buf[:, off : off + fd],
                    in0=pair_buf[:, 2 * off : 2 * off + fd],
                    scalar=0.0,
                    in1=pair_buf[:, 2 * off + fd : 2 * (off + fd)],
                    op0=mybir.AluOpType.bypass,
                    op1=mybir.AluOpType.is_gt,
                    accum_out=partials[:, k : k + 1],
                ).then_inc(dve_sem, 1)

    _strip_const_preamble(nc)
    return nc


def _strip_const_preamble(nc):
    """Bass.__init__ unconditionally materializes four const SBUF tensors
    (gpsimd memsets) and an all-engine barrier before main. This kernel
    reads none of the consts and has no cross-engine dependency at start
    (every data hand-off goes through explicit semaphores), so drop them
    from the entry block - worth ~0.8us of program preamble."""
    main_bb = nc.main_func.blocks[0]
    assert main_bb.name == "main"

    def removable(ins):
        t = type(ins).__name__
        if t == "InstMemset":
            return getattr(ins.outs[0], "memref", "").startswith("const-")
        return t in ("InstDrain", "InstEventSemaphore")

    main_bb.instructions[:] = [
        ins for ins in main_bb.instructions if not removable(ins)
    ]


def _make_pairs(synonymy_score, antonymy_score):
    """Build the per-core flat pair tensor: consecutive [128, 2, fd_j]
    blocks with ant rows first (in0), then syn rows (in1)."""
    syn = np.asarray(synonymy_score, dtype=np.float32).reshape(
        N_CORES, P, FD_TOTAL
    )
    ant = np.asarray(antonymy_score, dtype=np.float32).reshape(
        N_CORES, P, FD_TOTAL
    )
    blocks = []
    for k in range(N_CHUNKS):
        s, e = CHUNK_OFFS[k], CHUNK_OFFS[k + 1]
        blk = np.stack([ant[:, :, s:e], syn[:, :, s:e]], axis=2)  # [C,P,2,fd]
        blocks.append(blk.reshape(N_CORES, -1))
    return np.ascontiguousarray(np.concatenate(blocks, axis=1))  # [C, 2*PER_CORE]


def run(inputs, trace=False, trace_cores=None):
    """Run the SPMD kernel on 8 cores. Returns (result_scalar, BassKernelResults)."""
    global _NC
    if _NC is None:
        _NC = build_nc()

    pairs = _make_pairs(inputs["synonymy_score"], inputs["antonymy_score"])
    in_maps = [{"pairs": pairs[c]} for c in range(N_CORES)]
    bkr = run_bass_kernel_spmd(
        _NC,
        in_maps,
        list(range(N_CORES)),
        trace=trace,
        trace_cores=trace_cores,
    )
    total = sum(
        np.asarray(r["out"], dtype=np.float64).sum() for r in bkr.results
    )
    result = np.float32(-(total / B))
    return result, bkr


def kernel(S1_out, synonymy_score, antonymy_score):
    result, _ = run(
        {"synonymy_score": synonymy_score, "antonymy_score": antonymy_score}
    )
    return result


# revision 19
# speedup vs baseline: 1.2607x; 1.2607x over previous
"""Trainium2 Bass kernel for -mean(antonymy_score > synonymy_score).

Strategy: pure data-parallel over 8 NeuronCores. Each core receives a
contiguous 1/8 slice of the batch. On the host, antonymy/synonymy are
interleaved into one flat "pairs" tensor laid out as consecutive
[128, 2, fd_j] chunk blocks so each chunk arrives in a single contiguous
DMA carrying one semaphore (the walrus codegen path here allows only one
sync wait per instruction). Chunks alternate between the two HWDGE rings
(SP + ACT) so the two streams transfer concurrently (~341 GB/s observed,
~358 GB/s per-core HBM cap). Chunk sizes taper (2MB, 1MB, 0.5MB, 0.5MB
per ring) so the DVE work left after the last chunk lands is minimal.

Each chunk is consumed by one fused DVE scalar_tensor_tensor:
mask = (ant bypass 0) is_gt syn, with accum_out producing the
per-partition free-dim sum in the same pass. The [128, n_chunks] partial
counts are DMA'd back and the host computes -total/B (exact:
integer-valued fp32 counts). S1_out is unused by the computation - it
only fixes the batch size.

Raw Bass (no TileContext) keeps the program pre/postamble to a single
all-engine barrier instead of Tile's ~10us of drains + EVSEM
butterflies. One semaphore per chunk DMA: two in-flight DMAs sharing a
sem can interleave their 16 per-SDMA-engine increments, so a cumulative
wait could fire with the earlier chunk still incomplete.
"""

from contextlib import ExitStack

import numpy as np

import concourse.bass as bass
import concourse.mybir as mybir
from concourse.bass_utils import run_bass_kernel_spmd

B = 8388608
N_CORES = 8
PER_CORE = B // N_CORES  # 1048576
P = 128
FD_TOTAL = PER_CORE // P  # 8192 per array per core

# Per-chunk (free-dim size, stream) in the order the DVE consumes them.
# Three concurrent DMA streams: the two HWDGE rings (SP + ACT) and one
# SWDGE stream (gpsimd) - the two HWDGE rings share descriptor-gen
# hardware and one of them starts ~3us late (randomly which), so SWDGE
# provides a third independent stream. Each stream tapers to a small
# final chunk so little DVE work is exposed after its stream drains.
CHUNK_PLAN = [
    (256, "sp"),
    (256, "act"),
    (2048, "sp"),
    (2048, "act"),
    (1024, "sp"),
    (1024, "act"),
    (512, "sp"),
    (512, "act"),
    (256, "sp"),
    (256, "act"),
]
CHUNK_FDS = [fd for fd, _ in CHUNK_PLAN]
assert sum(CHUNK_FDS) == FD_TOTAL
N_CHUNKS = len(CHUNK_FDS)
CHUNK_OFFS = np.concatenate([[0], np.cumsum(CHUNK_FDS)]).tolist()

F32 = mybir.dt.float32

_NC = None


def build_nc():
    nc = bass.Bass()
    pairs = nc.dram_tensor("pairs", [2 * PER_CORE], F32, kind="ExternalInput")
    out = nc.dram_tensor("out", [P, N_CHUNKS], F32, kind="ExternalOutput")

    with ExitStack() as ctx:
        pair_buf = ctx.enter_context(
            nc.sbuf_tensor("pair_buf", [P, 2 * FD_TOTAL], F32)
        )
        mask_buf = ctx.enter_context(nc.sbuf_tensor("mask_buf", [P, FD_TOTAL], F32))
        partials = ctx.enter_context(nc.sbuf_tensor("partials", [P, N_CHUNKS], F32))
        chunk_sems = [
            ctx.enter_context(nc.semaphore(f"chunk{k}")) for k in range(N_CHUNKS)
        ]
        dve_sem = ctx.enter_context(nc.semaphore("dve_sem"))
        out_sem = ctx.enter_context(nc.semaphore("out_sem"))
        block = ctx.enter_context(nc.Block())

        def chunk_dma(eng, k):
            fd = CHUNK_FDS[k]
            off = CHUNK_OFFS[k]
            src = bass.AP(pairs, 2 * P * off, [[2 * fd, P], [1, 2 * fd]])
            dst = pair_buf[:, 2 * off : 2 * (off + fd)]
            eng.dma_start(dst, src).then_inc(chunk_sems[k], 16)

        @block.sync
        def _(sync: bass.BassEngine):
            for k in range(N_CHUNKS):
                if CHUNK_PLAN[k][1] == "sp":
                    chunk_dma(sync, k)
            sync.wait_ge(dve_sem, N_CHUNKS)
            sync.dma_start(out[:], partials[:]).then_inc(out_sem, 16)

        @block.scalar
        def _(scalar: bass.BassEngine):
            for k in range(N_CHUNKS):
                if CHUNK_PLAN[k][1] == "act":
                    chunk_dma(scalar, k)



        @block.vector
        def _(vector: bass.BassEngine):
            for k in range(N_CHUNKS):
                fd = CHUNK_FDS[k]
                off = CHUNK_OFFS[k]
                vector.wait_ge(chunk_sems[k], 16)
                # mask = (ant bypass 0.0) is_gt syn -> 1.0/0.0
                # partials[:, k] = free-dim sum of mask (same instruction)
                vector.scalar_tensor_tensor(
                    out=mask_buf[:, off : off + fd],
                    in0=pair_buf[:, 2 * off : 2 * off + fd],
                    scalar=0.0,
                    in1=pair_buf[:, 2 * off + fd : 2 * (off + fd)],
                    op0=mybir.AluOpType.bypass,
                    op1=mybir.AluOpType.is_gt,
                    accum_out=partials[:, k : k + 1],
                ).then_inc(dve_sem, 1)

    _strip_const_preamble(nc)
    return nc


def _strip_const_preamble(nc):
    """Bass.__init__ unconditionally materializes four const SBUF tensors
    (gpsimd memsets) and an all-engine barrier before main. This kernel
    reads none of the consts and has no cross-engine dependency at start
    (every data hand-off goes through explicit semaphores), so drop them
    from the entry block - worth ~0.8us of program preamble."""
    main_bb = nc.main_func.blocks[0]
    assert main_bb.name == "main"

    def removable(ins):
        t = type(ins).__name__
        if t == "InstMemset":
            return getattr(ins.outs[0], "memref", "").startswith("const-")
        return t in ("InstDrain", "InstEventSemaphore")

    main_bb.instructions[:] = [
        ins for ins in main_bb.instructions if not removable(ins)
    ]


def _make_pairs(synonymy_score, antonymy_score):
    """Build the per-core flat pair tensor: consecutive [128, 2, fd_j]
    blocks with ant rows first (in0), then syn rows (in1)."""
    syn = np.asarray(synonymy_score, dtype=np.float32).reshape(
        N_CORES, P, FD_TOTAL
    )
    ant = np.asarray(antonymy_score, dtype=np.float32).reshape(
        N_CORES, P, FD_TOTAL
    )
    blocks = []
    for k in range(N_CHUNKS):
        s, e = CHUNK_OFFS[k], CHUNK_OFFS[k + 1]
        blk = np.stack([ant[:, :, s:e], syn[:, :, s:e]], axis=2)  # [C,P,2,fd]
        blocks.append(blk.reshape(N_CORES, -1))
    return np.ascontiguousarray(np.concatenate(blocks, axis=1))  # [C, 2*PER_CORE]


def run(inputs, trace=False, trace_cores=None):
    """Run the SPMD kernel on 8 cores. Returns (result_scalar, BassKernelResults)."""
    global _NC
    if _NC is None:
        _NC = build_nc()

    pairs = _make_pairs(inputs["synonymy_score"], inputs["antonymy_score"])
    in_maps = [{"pairs": pairs[c]} for c in range(N_CORES)]
    bkr = run_bass_kernel_spmd(
        _NC,
        in_maps,
        list(range(N_CORES)),
        trace=trace,
        trace_cores=trace_cores,
    )
    total = sum(
        np.asarray(r["out"], dtype=np.float64).sum() for r in bkr.results
    )
    result = np.float32(-(total / B))
    return result, bkr


def kernel(S1_out, synonymy_score, antonymy_score):
    result, _ = run(
        {"synonymy_score": synonymy_score, "antonymy_score": antonymy_score}
    )
    return result
